# revision 16
# baseline (speedup 1.0000x reference)
"""Masked mean-pooling (nn_MaskedPooling) Trainium2 Bass kernel.

Reference semantics (jax):
    keep   = (~mask).astype(f32)               # [B, T]
    denom  = keep.sum(axis=1)                  # [B]
    out    = einsum('btd,bt->bd', x, keep) / denom[:, None]

Shapes: x [32, 4096, 512] f32, mask [32, 4096] bool -> out [32, 512] f32.
Data-parallel over batch: 8 NeuronCores x 4 examples.

Structure (both impls share it):
  * T is split as t = p*32 + n (p = SBUF partition, n = chunk column), so
    the keep matrix loads directly in the layout the PE needs.
  * The masked sum over T is a PE matmul per T-chunk: the keep chunk
    ([128, 1] stationary operand, f32r single-pass) contracts with the x
    chunk [128, 512], accumulating over chunks in PSUM.
  * Denominators come from one matmul with a ones-vector against the keep
    matrix, then a free-dim reduce + reciprocal.

Implementations:
  * "indirect" (default): the mask drops ~50% of rows, so streaming all
    of x (32 MiB/core, ~94 us at 358 GB/s HBM) wastes half the traffic.
    The x tiles are fetched with SWDGE *indirect* DMA instead: the
    per-row offset list is iota + mask*2^20, with bounds_check=rows-1 and
    oob_is_err=False, so masked rows are silently skipped - no descriptor,
    no HBM read.  Skipped slots hold stale SBUF (or zeros, depending on
    HW OOB semantics); either way the keep[t]=0 stationary weight zeroes
    their contribution exactly.  The first-cycle tile buffers are memset
    so stale SBUF is never NaN/Inf (0.0 * NaN would poison PSUM).
    HBM traffic drops to ~kept * 2 KiB (~17 MiB/core, ~47 us floor).
  * "dense": stream all of x with plain SWDGE DMAs (the ~94 us
    memory-bound roofline version); MP_IMPL=dense selects it.
"""

import os
from contextlib import ExitStack

import numpy as np

import concourse.bass as bass
import concourse.mybir as mybir
import concourse.tile as tile
from concourse import bacc, bass_utils
from concourse.bass import IndirectOffsetOnAxis

B, T, D = 32, 4096, 512
N_CORES = 8
BS = B // N_CORES  # examples per core
P = 128  # SBUF partitions
NCHUNK = T // P  # T-chunks per example (32)
OOB_BIG = 1 << 20  # added to masked rows' offsets -> fails bounds check

IMPL = os.environ.get("MP_IMPL", "dense")
MM_DTYPE = os.environ.get("MP_MM_DTYPE", "f32r")
X_BUFS = int(os.environ.get("MP_X_BUFS", "5"))
# dense: 16 chunks -> 4 MiB DMAs; indirect: 8 chunks -> 1024-entry lists
CHUNKS_PER_TILE = int(
    os.environ.get("MP_CHUNKS_PER_TILE", "8" if IMPL == "indirect" else "16")
)
N_DMA_ENGINES = int(os.environ.get("MP_DMA_ENGINES", "0"))
# Per-example tile schedule (chunk counts, must sum to NCHUNK). A tapered
# tail keeps the PE drain after the last DMA byte short.
SEGS = [int(s) for s in os.environ.get("MP_SEGS", "16,16").split(",")]


def iota_np():
    # iota[p, b, n] = b*T + p*NCHUNK + n : each example's global row ids in
    # the (partition, chunk) layout. Constant (data-independent).
    t_local = np.arange(T, dtype=np.int32).reshape(P, NCHUNK)
    return np.stack([b * T + t_local for b in range(BS)], axis=1).copy()


def build_bass(
    impl=IMPL,
    bs=BS,
    t=T,
    d=D,
    chunks_per_tile=CHUNKS_PER_TILE,
    x_bufs=X_BUFS,
    mm_dtype=MM_DTYPE,
    n_cores=N_CORES,
    n_dma_engines=N_DMA_ENGINES,
):
    nchunk = t // P
    assert t % P == 0 and nchunk % chunks_per_tile == 0
    # Bacc (not raw Bass): its compile() pass splits multi-semaphore waits
    # into event-semaphore chains - walrus accepts at most one sync wait
    # per instruction.
    nc = bacc.Bacc(
        trn_type="TRN2",
        target_bir_lowering=False,
        debug=False,
        num_devices=n_cores,
    )
    # float32r is bit-identical to float32 in memory; declaring the tensors
    # as f32r end-to-end satisfies the BIR verifier's "producer must round
    # to FP32r" rule with plain copies.
    mmdt = mybir.dt.float32r if mm_dtype == "f32r" else mybir.dt.float32
    x = nc.dram_tensor("x", [bs, t, d], mmdt, kind="ExternalInput").ap()
    mask = nc.dram_tensor("mask", [bs, t], mybir.dt.uint8, kind="ExternalInput").ap()
    if impl == "indirect":
        iota = nc.dram_tensor(
            "iota", [P, bs, nchunk], mybir.dt.int32, kind="ExternalInput"
        ).ap()
    out = nc.dram_tensor("out", [bs, d], mybir.dt.float32, kind="ExternalOutput").ap()

    with tile.TileContext(nc) as tc, ExitStack() as ctx:
        singles = ctx.enter_context(tc.tile_pool(name="singles", bufs=1))
        xpool = ctx.enter_context(tc.tile_pool(name="xpool", bufs=x_bufs))
        tails = ctx.enter_context(tc.tile_pool(name="tails", bufs=4))
        psum = ctx.enter_context(tc.tile_pool(name="psum", bufs=1, space="PSUM"))
        accs = ctx.enter_context(tc.tile_pool(name="accs", bufs=4, space="PSUM"))

        # ones vector for the denominator matmul.
        ones = singles.tile([P, 1], mmdt)
        if mmdt == mybir.dt.float32r:
            # Memset can't target f32r; produce via DVE copy (the "rounding"
            # producer the BIR verifier wants).
            ones_f32 = singles.tile([P, 1], mybir.dt.float32)
            nc.vector.memset(ones_f32, 1.0)
            nc.vector.tensor_copy(out=ones, in_=ones_f32)
        else:
            nc.vector.memset(ones, 1.0)

        # Mask loads directly in lhsT layout: m_u8[p, j] = mask[b, p*32 + n]
        m_u8 = singles.tile([P, bs, nchunk], mybir.dt.uint8)
        nc.sync.dma_start(out=m_u8, in_=mask.rearrange("b (p n) -> p b n", p=P))
        m_f = singles.tile([P, bs, nchunk], mybir.dt.float32)
        nc.vector.tensor_copy(out=m_f, in_=m_u8)
        # keep = 1 - m
        keep = singles.tile([P, bs, nchunk], mmdt)
        nc.vector.tensor_scalar(
            out=keep,
            in0=m_f,
            scalar1=-1.0,
            scalar2=1.0,
            op0=mybir.AluOpType.mult,
            op1=mybir.AluOpType.add,
        )

        if impl == "indirect":
            # Row-offset list: idx = iota + m*2^20. Kept rows carry their
            # global row id; masked rows fail the bounds check and are
            # skipped by the SWDGE descriptor generator.
            iota_sb = singles.tile([P, bs, nchunk], mybir.dt.int32)
            nc.sync.dma_start(out=iota_sb, in_=iota)
            m_i32 = singles.tile([P, bs, nchunk], mybir.dt.int32)
            nc.vector.tensor_copy(out=m_i32, in_=m_u8)
            idx = singles.tile([P, bs, nchunk], mybir.dt.int32)
            nc.vector.scalar_tensor_tensor(
                out=idx,
                in0=m_i32,
                scalar=float(OOB_BIG),
                in1=iota_sb,
                op0=mybir.AluOpType.mult,
                op1=mybir.AluOpType.add,
            )
            x_flat = x.rearrange("b t d -> (b t) d")

        # Denominators: den[j] = sum_p keep[p, j]; reduce chunks per example.
        den_ps = psum.tile([1, bs, nchunk], mybir.dt.float32)
        nc.tensor.matmul(den_ps, ones, keep, start=True, stop=True)
        den = tails.tile([1, bs], mybir.dt.float32)
        nc.vector.tensor_reduce(
            out=den,
            in_=den_ps,
            axis=mybir.AxisListType.X,
            op=mybir.AluOpType.add,
        )
        rec = tails.tile([1, bs], mybir.dt.float32)
        nc.vector.reciprocal(rec, den)

        if n_dma_engines == 0:
            dma_engines = [nc.gpsimd]
            out_dma = nc.sync
        else:
            dma_engines = [nc.sync, nc.scalar][:n_dma_engines]
            out_dma = nc.gpsimd

        if impl == "indirect":
            segs = [chunks_per_tile] * (nchunk // chunks_per_tile)
        else:
            segs = SEGS
            assert sum(segs) == nchunk, segs

        dma_i = 0
        xtile_i = 0
        for b in range(bs):
            x_b = x[b].rearrange("(p n) d -> p n d", p=P)
            acc_ps = accs.tile([1, d], mybir.dt.float32)
            n0 = 0
            for seg in segs:
                x_tile = xpool.tile([P, seg, d], mmdt, tag="x_tile")
                if impl == "indirect":
                    if xtile_i < x_bufs:
                        # First use of this buffer: skipped rows would
                        # otherwise read cold SBUF, which may be NaN/Inf -
                        # keep=0.0 would not neutralize that.
                        nc.vector.memset(x_tile.bitcast(mybir.dt.uint32), 0)
                    nc.gpsimd.indirect_dma_start(
                        out=x_tile,
                        out_offset=None,
                        in_=x_flat,
                        in_offset=IndirectOffsetOnAxis(
                            ap=idx[:, b, n0 : n0 + seg], axis=0
                        ),
                        bounds_check=bs * t - 1,
                        oob_is_err=False,
                    )
                else:
                    dma_engines[dma_i % len(dma_engines)].dma_start(
                        out=x_tile,
                        in_=x_b[:, n0 : n0 + seg, :],
                    )
                    dma_i += 1
                xtile_i += 1
                for k in range(seg):
                    n = n0 + k
                    nc.tensor.matmul(
                        acc_ps,
                        keep[:, b, n : n + 1],
                        x_tile[:, k, :],
                        start=(n == 0),
                        stop=(n == nchunk - 1),
                    )
                n0 += seg
            # out[b] = acc / denom[b]
            o_sb = tails.tile([1, d], mybir.dt.float32)
            nc.vector.tensor_scalar_mul(o_sb, acc_ps, rec[0:1, b : b + 1])
            out_dma.dma_start(out=out[b : b + 1, :], in_=o_sb)

    nc.finalize()
    return nc


def prepare(x: np.ndarray, mask: np.ndarray):
    """Build the Bass kernel and shard the inputs.

    Returns (nc, in_maps, impl_name)."""
    assert x.shape == (B, T, D) and mask.shape == (B, T)
    impl = IMPL
    nc = build_bass(impl=impl)
    mask_u8 = np.ascontiguousarray(mask).view(np.uint8)
    iota = iota_np() if impl == "indirect" else None
    in_maps = []
    for i in range(N_CORES):
        m = {
            "x": np.ascontiguousarray(x[i * BS : (i + 1) * BS]),
            "mask": np.ascontiguousarray(mask_u8[i * BS : (i + 1) * BS]),
        }
        if iota is not None:
            m["iota"] = iota
        in_maps.append(m)
    return nc, in_maps, impl


def kernel(x: np.ndarray, mask: np.ndarray) -> np.ndarray:
    nc, in_maps, _ = prepare(x, mask)
    res = bass_utils.run_bass_kernel_spmd(nc, in_maps, core_ids=list(range(N_CORES)))
    out = np.concatenate([r["out"] for r in res.results], axis=0)
    return out.astype(np.float32, copy=False)


# revision 19
# speedup vs baseline: 1.2877x; 1.2877x over previous
"""Masked mean-pooling (nn_MaskedPooling) Trainium2 Bass kernel.

Reference semantics (jax):
    keep   = (~mask).astype(f32)               # [B, T]
    denom  = keep.sum(axis=1)                  # [B]
    out    = einsum('btd,bt->bd', x, keep) / denom[:, None]

Shapes: x [32, 4096, 512] f32, mask [32, 4096] bool -> out [32, 512] f32.
Data-parallel over batch: 8 NeuronCores x 4 examples.

Structure (both impls share it):
  * T is split as t = p*32 + n (p = SBUF partition, n = chunk column), so
    the keep matrix loads directly in the layout the PE needs.
  * The masked sum over T is a PE matmul per T-chunk: the keep chunk
    ([128, 1] stationary operand, f32r single-pass) contracts with the x
    chunk [128, 512], accumulating over chunks in PSUM.
  * Denominators come from one matmul with a ones-vector against the keep
    matrix, then a free-dim reduce + reciprocal.

Implementations:
  * "indirect" (default): the mask drops ~50% of rows, so streaming all
    of x (32 MiB/core, ~94 us at 358 GB/s HBM) wastes half the traffic.
    The x tiles are fetched with SWDGE *indirect* DMA instead: the
    per-row offset list is iota + mask*2^20, with bounds_check=rows-1 and
    oob_is_err=False, so masked rows are silently skipped - no descriptor,
    no HBM read.  Skipped slots hold stale SBUF (or zeros, depending on
    HW OOB semantics); either way the keep[t]=0 stationary weight zeroes
    their contribution exactly.  The first-cycle tile buffers are memset
    so stale SBUF is never NaN/Inf (0.0 * NaN would poison PSUM).
    HBM traffic drops to ~kept * 2 KiB (~17 MiB/core, ~47 us floor).
  * "dense": stream all of x with plain SWDGE DMAs (the ~94 us
    memory-bound roofline version); MP_IMPL=dense selects it.
"""

import os
from contextlib import ExitStack

import numpy as np

import concourse.bass as bass
import concourse.mybir as mybir
import concourse.tile as tile
from concourse import bacc, bass_utils
from concourse.bass import IndirectOffsetOnAxis

B, T, D = 32, 4096, 512
N_CORES = 8
BS = B // N_CORES  # examples per core
P = 128  # SBUF partitions
NCHUNK = T // P  # T-chunks per example (32)
OOB_BIG = 1 << 20  # added to masked rows' offsets -> fails bounds check

IMPL = os.environ.get("MP_IMPL", "dense")
MM_DTYPE = os.environ.get("MP_MM_DTYPE", "f32r")
X_BUFS = int(os.environ.get("MP_X_BUFS", "5"))
# dense: 16 chunks -> 4 MiB DMAs; indirect: 8 chunks -> 1024-entry lists
CHUNKS_PER_TILE = int(
    os.environ.get("MP_CHUNKS_PER_TILE", "8" if IMPL == "indirect" else "16")
)
N_DMA_ENGINES = int(os.environ.get("MP_DMA_ENGINES", "0"))
# Per-example tile schedule (chunk counts, must sum to NCHUNK). The last
# example gets a tapered tail so the PE drain after the final DMA byte is
# short; earlier examples keep big cheap tiles.
SEGS = [int(s) for s in os.environ.get("MP_SEGS", "16,16").split(",")]
TAIL_SEGS = [int(s) for s in os.environ.get("MP_TAIL_SEGS", "16,12,4").split(",")]
# Leading x DMAs issued on Sync (HWDGE): the Q7/SWDGE stream can't start
# until the GpSimd prologue (~9 us) finishes, but Sync is ready at ~2.5 us.
HEAD_SYNC = int(os.environ.get("MP_HEAD_SYNC", "2"))


def iota_np():
    # iota[p, b, n] = b*T + p*NCHUNK + n : each example's global row ids in
    # the (partition, chunk) layout. Constant (data-independent).
    t_local = np.arange(T, dtype=np.int32).reshape(P, NCHUNK)
    return np.stack([b * T + t_local for b in range(BS)], axis=1).copy()


def build_bass(
    impl=IMPL,
    bs=BS,
    t=T,
    d=D,
    chunks_per_tile=CHUNKS_PER_TILE,
    x_bufs=X_BUFS,
    mm_dtype=MM_DTYPE,
    n_cores=N_CORES,
    n_dma_engines=N_DMA_ENGINES,
):
    nchunk = t // P
    assert t % P == 0 and nchunk % chunks_per_tile == 0
    # Bacc (not raw Bass): its compile() pass splits multi-semaphore waits
    # into event-semaphore chains - walrus accepts at most one sync wait
    # per instruction.
    nc = bacc.Bacc(
        trn_type="TRN2",
        target_bir_lowering=False,
        debug=False,
        num_devices=n_cores,
    )
    # float32r is bit-identical to float32 in memory; declaring the tensors
    # as f32r end-to-end satisfies the BIR verifier's "producer must round
    # to FP32r" rule with plain copies.
    mmdt = mybir.dt.float32r if mm_dtype == "f32r" else mybir.dt.float32
    x = nc.dram_tensor("x", [bs, t, d], mmdt, kind="ExternalInput").ap()
    mask = nc.dram_tensor("mask", [bs, t], mybir.dt.uint8, kind="ExternalInput").ap()
    if impl == "indirect":
        iota = nc.dram_tensor(
            "iota", [P, bs, nchunk], mybir.dt.int32, kind="ExternalInput"
        ).ap()
    out = nc.dram_tensor("out", [bs, d], mybir.dt.float32, kind="ExternalOutput").ap()

    with tile.TileContext(nc) as tc, ExitStack() as ctx:
        singles = ctx.enter_context(tc.tile_pool(name="singles", bufs=1))
        xpool = ctx.enter_context(tc.tile_pool(name="xpool", bufs=x_bufs))
        tails = ctx.enter_context(tc.tile_pool(name="tails", bufs=4))
        psum = ctx.enter_context(tc.tile_pool(name="psum", bufs=1, space="PSUM"))
        accs = ctx.enter_context(tc.tile_pool(name="accs", bufs=4, space="PSUM"))

        # ones vector for the denominator matmul.
        ones = singles.tile([P, 1], mmdt)
        if mmdt == mybir.dt.float32r:
            # Memset can't target f32r; produce via DVE copy (the "rounding"
            # producer the BIR verifier wants).
            ones_f32 = singles.tile([P, 1], mybir.dt.float32)
            nc.vector.memset(ones_f32, 1.0)
            nc.vector.tensor_copy(out=ones, in_=ones_f32)
        else:
            nc.vector.memset(ones, 1.0)

        # Mask loads directly in lhsT layout: m_u8[p, j] = mask[b, p*32 + n]
        m_u8 = singles.tile([P, bs, nchunk], mybir.dt.uint8)
        nc.sync.dma_start(out=m_u8, in_=mask.rearrange("b (p n) -> p b n", p=P))
        m_f = singles.tile([P, bs, nchunk], mybir.dt.float32)
        nc.vector.tensor_copy(out=m_f, in_=m_u8)
        # keep = 1 - m
        keep = singles.tile([P, bs, nchunk], mmdt)
        nc.vector.tensor_scalar(
            out=keep,
            in0=m_f,
            scalar1=-1.0,
            scalar2=1.0,
            op0=mybir.AluOpType.mult,
            op1=mybir.AluOpType.add,
        )

        if impl == "indirect":
            # Row-offset list: idx = iota + m*2^20. Kept rows carry their
            # global row id; masked rows fail the bounds check and are
            # skipped by the SWDGE descriptor generator.
            iota_sb = singles.tile([P, bs, nchunk], mybir.dt.int32)
            nc.sync.dma_start(out=iota_sb, in_=iota)
            m_i32 = singles.tile([P, bs, nchunk], mybir.dt.int32)
            nc.vector.tensor_copy(out=m_i32, in_=m_u8)
            idx = singles.tile([P, bs, nchunk], mybir.dt.int32)
            nc.vector.scalar_tensor_tensor(
                out=idx,
                in0=m_i32,
                scalar=float(OOB_BIG),
                in1=iota_sb,
                op0=mybir.AluOpType.mult,
                op1=mybir.AluOpType.add,
            )
            x_flat = x.rearrange("b t d -> (b t) d")

        # Denominators: den[j] = sum_p keep[p, j]; reduce chunks per example.
        den_ps = psum.tile([1, bs, nchunk], mybir.dt.float32)
        nc.tensor.matmul(den_ps, ones, keep, start=True, stop=True)
        den = tails.tile([1, bs], mybir.dt.float32)
        nc.vector.tensor_reduce(
            out=den,
            in_=den_ps,
            axis=mybir.AxisListType.X,
            op=mybir.AluOpType.add,
        )
        rec = tails.tile([1, bs], mybir.dt.float32)
        nc.vector.reciprocal(rec, den)

        if n_dma_engines == 0:
            dma_engines = [nc.gpsimd]
            out_dma = nc.sync
        else:
            dma_engines = [nc.sync, nc.scalar][:n_dma_engines]
            out_dma = nc.gpsimd

        def segs_for(b):
            if impl == "indirect":
                return [chunks_per_tile] * (nchunk // chunks_per_tile)
            s = TAIL_SEGS if b == bs - 1 else SEGS
            assert sum(s) == nchunk, s
            return s

        dma_i = 0
        xtile_i = 0
        for b in range(bs):
            x_b = x[b].rearrange("(p n) d -> p n d", p=P)
            acc_ps = accs.tile([1, d], mybir.dt.float32)
            n0 = 0
            for seg in segs_for(b):
                x_tile = xpool.tile([P, seg, d], mmdt, tag="x_tile")
                if impl == "indirect":
                    if xtile_i < x_bufs:
                        # First use of this buffer: skipped rows would
                        # otherwise read cold SBUF, which may be NaN/Inf -
                        # keep=0.0 would not neutralize that.
                        nc.vector.memset(x_tile.bitcast(mybir.dt.uint32), 0)
                    nc.gpsimd.indirect_dma_start(
                        out=x_tile,
                        out_offset=None,
                        in_=x_flat,
                        in_offset=IndirectOffsetOnAxis(
                            ap=idx[:, b, n0 : n0 + seg], axis=0
                        ),
                        bounds_check=bs * t - 1,
                        oob_is_err=False,
                    )
                else:
                    if dma_i < HEAD_SYNC and n_dma_engines == 0:
                        eng = nc.sync
                    else:
                        eng = dma_engines[dma_i % len(dma_engines)]
                    eng.dma_start(
                        out=x_tile,
                        in_=x_b[:, n0 : n0 + seg, :],
                    )
                    dma_i += 1
                xtile_i += 1
                for k in range(seg):
                    n = n0 + k
                    nc.tensor.matmul(
                        acc_ps,
                        keep[:, b, n : n + 1],
                        x_tile[:, k, :],
                        start=(n == 0),
                        stop=(n == nchunk - 1),
                    )
                n0 += seg
            # out[b] = acc / denom[b]
            o_sb = tails.tile([1, d], mybir.dt.float32)
            nc.vector.tensor_scalar_mul(o_sb, acc_ps, rec[0:1, b : b + 1])
            out_dma.dma_start(out=out[b : b + 1, :], in_=o_sb)

    nc.finalize()
    return nc


def prepare(x: np.ndarray, mask: np.ndarray):
    """Build the Bass kernel and shard the inputs.

    Returns (nc, in_maps, impl_name)."""
    assert x.shape == (B, T, D) and mask.shape == (B, T)
    impl = IMPL
    nc = build_bass(impl=impl)
    mask_u8 = np.ascontiguousarray(mask).view(np.uint8)
    iota = iota_np() if impl == "indirect" else None
    in_maps = []
    for i in range(N_CORES):
        m = {
            "x": np.ascontiguousarray(x[i * BS : (i + 1) * BS]),
            "mask": np.ascontiguousarray(mask_u8[i * BS : (i + 1) * BS]),
        }
        if iota is not None:
            m["iota"] = iota
        in_maps.append(m)
    return nc, in_maps, impl


def kernel(x: np.ndarray, mask: np.ndarray) -> np.ndarray:
    nc, in_maps, _ = prepare(x, mask)
    res = bass_utils.run_bass_kernel_spmd(nc, in_maps, core_ids=list(range(N_CORES)))
    out = np.concatenate([r["out"] for r in res.results], axis=0)
    return out.astype(np.float32, copy=False)
